# revision 18
# baseline (speedup 1.0000x reference)
"""GATv2 (2-layer) + linear head GNN kernel for Trainium2, 8 NeuronCores.

Strategy: nodes are permuted into degree-balanced blocks of 128; blocks are
sharded contiguously across 8 cores. Each core processes all edges whose
destination lies in its blocks (edges grouped by destination block on the
host). Segment softmax and scatter-add are block-local: per 128-edge chunk a
one-hot (edge x node) matrix is built on-device with an iota/is_equal compare
and used as a matmul operand, so PSUM accumulates the per-node exp-sums and
weighted feature sums. Self loops are handled by a per-block "loop chunk"
whose edge features are the on-device computed mean of incoming edge_attr.
Layer-2 source projections are exchanged with a single AllGather.
"""
import sys

sys.path.insert(0, "/opt/trn_rl_repo")

import numpy as np
import concourse.bass as bass
import concourse.mybir as mybir
import concourse.tile as tile
from concourse import bacc
from concourse.masks import make_identity

P = 128
HEADS = 4
HC = 32          # channels per head, layer 1
H1 = HEADS * HC  # 128
C2 = 8
EDIM = 16
OUT = 8
NCORES = 8
NF = 24          # rec fields: [ex*4 | ea*16 | one | srcp_i32 | dstf | dstloc_i32]
F_EX = 0
F_EA = 4
F_ONE = 20
F_SRC = 21
F_DST = 22
F_DLOC = 23
PAD_DST = 999.0

FP = mybir.dt.float32
I32 = mybir.dt.int32


# --------------------------------------------------------------------------
# host-side preprocessing
# --------------------------------------------------------------------------

def balanced_blocks(deg, n_pad):
    """Assign each node (0..n_pad-1) to a block of exactly P nodes so block
    degree sums are balanced. Returns permpos[node] = block*P + slot."""
    import heapq

    nb = n_pad // P
    order = np.argsort(-deg, kind="stable")
    heap = [(0, b) for b in range(nb)]
    heapq.heapify(heap)
    counts = np.zeros(nb, np.int64)
    permpos = np.empty(n_pad, np.int64)
    slot_of = np.zeros(nb, np.int64)
    for node in order:
        while True:
            s, b = heapq.heappop(heap)
            if counts[b] < P:
                break
        permpos[node] = b * P + slot_of[b]
        slot_of[b] += 1
        counts[b] += 1
        if counts[b] < P:
            heapq.heappush(heap, (s + int(deg[node]), b))
    return permpos


def prep(x, edge_index, edge_attr, npc, cpb=None):
    """Build permuted/padded inputs and per-core edge records."""
    n, din = x.shape
    e = edge_attr.shape[0]
    n_pad = NCORES * npc * P
    nb = n_pad // P
    src = np.asarray(edge_index[0], np.int64)
    dst = np.asarray(edge_index[1], np.int64)

    deg = np.bincount(dst, minlength=n_pad).astype(np.int64)
    permpos = balanced_blocks(deg, n_pad)

    orderv = np.empty(n_pad, np.int64)
    orderv[permpos] = np.arange(n_pad)
    xp = np.zeros((n_pad, din), np.float32)
    xp[permpos[:n]] = np.asarray(x, np.float32)

    blk = permpos // P        # block of each node
    slot = permpos % P

    eb = blk[dst]             # block of each edge
    # order edges by block
    eorder = np.argsort(eb, kind="stable")
    eb_s = eb[eorder]
    counts = np.bincount(eb_s, minlength=nb)
    need = int(np.ceil(counts.max() / P))
    if cpb is None:
        cpb = need
    assert cpb >= need, (cpb, need)

    rec = np.zeros((nb, P, cpb, NF), np.float32)
    rec[:, :, :, F_DST] = PAD_DST
    # positions within block
    starts = np.zeros(nb + 1, np.int64)
    np.cumsum(counts, out=starts[1:])
    pos_in_blk = np.arange(e) - starts[eb_s]
    cc = pos_in_blk // P
    pp = pos_in_blk % P
    es, ed = src[eorder], dst[eorder]
    rec[eb_s, pp, cc, F_SRC] = permpos[es].astype(np.int32).view(np.float32)
    rec[eb_s, pp, cc, F_DST] = slot[ed].astype(np.float32)
    rec[eb_s, pp, cc, F_DLOC] = (
        ((eb_s % npc) * P + slot[ed]).astype(np.int32).view(np.float32)
    )
    rec[eb_s, pp, cc, F_EA : F_EA + EDIM] = np.asarray(edge_attr, np.float32)[eorder]
    rec[eb_s, pp, cc, F_ONE] = 1.0
    # pad slots: srcp/dstloc default 0 (valid), dstf=999 disables them

    rd = 1.0 / np.maximum(deg.astype(np.float32), 1.0)
    rdp = np.empty(n_pad, np.float32)
    rdp[permpos] = rd
    # rdt per core: [P, npc]
    rdt = rdp.reshape(nb, P).transpose(1, 0)  # [P, nb]

    return dict(
        xp=xp, rec=rec, rdt=rdt, permpos=permpos, orderv=orderv,
        n_pad=n_pad, nb=nb, cpb=cpb,
    )


def prep_weights(w):
    """Host-side packing of model weights."""
    W1l = np.asarray(w["W1l"], np.float32)
    W1r = np.asarray(w["W1r"], np.float32)
    We1a = np.concatenate(
        [np.asarray(w["We1"], np.float32),
         (np.asarray(w["b1l"]) + np.asarray(w["b1r"])).astype(np.float32)[None, :]],
        axis=0,
    )  # [17, H1]
    att1 = np.asarray(w["att1"], np.float32)        # [HEADS, HC]
    attB = np.zeros((P, H1), np.float32)
    attB[:] = att1.reshape(-1)[None, :]             # [P, 128] replicated rows
    E4 = np.zeros((HEADS, H1), np.float32)
    for h in range(HEADS):
        E4[h, h * HC : (h + 1) * HC] = 1.0
    bias1e = (np.asarray(w["bias1"]) + np.asarray(w["b1l"])).astype(np.float32)[:, None]

    W2l = np.asarray(w["W2l"], np.float32)
    W2r = np.asarray(w["W2r"], np.float32)
    We2a = np.concatenate(
        [np.asarray(w["We2"], np.float32),
         (np.asarray(w["b2l"]) + np.asarray(w["b2r"])).astype(np.float32)[None, :]],
        axis=0,
    )  # [17, C2]
    att2r = np.zeros((P, C2), np.float32)
    att2r[:] = np.asarray(w["att2"], np.float32).reshape(-1)[None, :]
    bias2e = (np.asarray(w["bias2"]) + np.asarray(w["b2l"])).astype(np.float32)[:, None]
    ones18 = np.ones((1, C2), np.float32)
    Wlin = np.asarray(w["Wlin"], np.float32)
    blin = np.asarray(w["blin"], np.float32)[:, None]
    return dict(
        W1l=W1l, W1r=W1r, We1a=We1a, attB=attB, E4=E4, bias1e=bias1e,
        W2l=W2l, W2r=W2r, We2a=We2a, att2r=att2r, bias2e=bias2e,
        ones18=ones18, Wlin=Wlin, blin=blin,
    )


# --------------------------------------------------------------------------
# numpy emulator of the device algorithm (for debugging, not used by kernel)
# --------------------------------------------------------------------------

def np_forward(pp, wp, npc):
    xp, rec, rdt = pp["xp"], pp["rec"], pp["rdt"]
    nb, _, cpb, _ = rec.shape
    n_pad = pp["n_pad"]
    rec = rec.copy()

    def lrelu(v):
        return np.where(v > 0, v, 0.2 * v)

    def elu(v):
        return np.where(v > 0, v, np.exp(np.minimum(v, 0)) - 1.0)

    xl2loc = np.zeros((n_pad, C2), np.float32)
    xr2loc = np.zeros((n_pad, C2), np.float32)
    easum_all = np.zeros((nb, 17, P), np.float32)

    # layer 1 per block
    hT_all = np.zeros((nb, H1, P), np.float32)
    for b in range(nb):
        r = rec[b]                      # [P, cpb, NF]
        srcp = r[:, :, F_SRC].view(np.int32)
        dstf = r[:, :, F_DST]
        oh = (dstf[:, :, None] == np.arange(P)[None, None, :]).astype(np.float32)
        xg = xp[srcp]                   # [P, cpb, 128]
        xown = xp[b * P : (b + 1) * P]  # [P, 128]
        xr = xown @ wp["W1r"]
        xl = xg @ wp["W1l"]             # [P, cpb, 128]
        eaa = r[:, :, F_EA : F_EA + 17]
        m = lrelu(xl + np.einsum("pcn,nf->pcf", oh, xr)
                  + eaa @ wp["We1a"])
        logits = (m * wp["attB"][0][None, None, :]).reshape(P, cpb, HEADS, HC).sum(-1)
        ex = np.exp(logits)             # [P, cpb, 4]
        r[:, :, F_EX : F_EX + 4] = ex
        xlw = xl * np.repeat(ex, HC, axis=2)
        numerT = np.einsum("pcf,pcn->fn", xlw, oh)
        meta = np.einsum("pcj,pcn->jn", r[:, :, 0:21], oh)  # [21, P]
        easum = meta[4:21]              # [17, P] (row 16 = deg)
        easum_all[b] = easum
        # loop chunk
        efd = (easum.T @ wp["We1a"]) * rdt[:, b][:, None]
        mloop = lrelu(xown @ wp["W1l"] + xr + efd)
        exl = np.exp(
            (mloop * wp["attB"][0][None, :]).reshape(P, HEADS, HC).sum(-1))
        denom = meta[0:4] + exl.T       # [4, P]
        numerT = numerT + ((xown @ wp["W1l"]) * np.repeat(exl, HC, 1)).T
        rfull = np.repeat(1.0 / denom, HC, axis=0)  # [128, P]
        hT = numerT * rfull + wp["bias1e"]
        hT = elu(hT)
        xl2loc[b * P : (b + 1) * P] = hT.T @ wp["W2l"]
        xr2loc[b * P : (b + 1) * P] = hT.T @ wp["W2r"]
        hT_all[b] = hT

    # layer 2 per block
    y = np.zeros((n_pad, OUT), np.float32)
    for b in range(nb):
        core = b // npc
        r = rec[b]
        srcp = r[:, :, F_SRC].view(np.int32)
        dloc = r[:, :, F_DLOC].view(np.int32)
        dstf = r[:, :, F_DST]
        oh = (dstf[:, :, None] == np.arange(P)[None, None, :]).astype(np.float32)
        xl2g = xl2loc[srcp]             # [P, cpb, 8]
        xr2g = xr2loc[core * npc * P + dloc]
        eaa = r[:, :, F_EA : F_EA + 17]
        m2 = lrelu(xl2g + xr2g + eaa @ wp["We2a"])
        logits2 = (m2 * wp["att2r"][0][None, None, :]).sum(-1)   # [P, cpb]
        ex2 = np.exp(logits2)
        xl2w = xl2g * ex2[:, :, None]
        meta2 = np.einsum(
            "pcj,pcn->jn",
            np.concatenate([xl2w, ex2[:, :, None]], axis=2), oh)  # [9, P]
        # loop
        xl2o = xl2loc[b * P : (b + 1) * P]
        xr2o = xr2loc[b * P : (b + 1) * P]
        ef2d = (easum_all[b].T @ wp["We2a"]) * rdt[:, b][:, None]
        m2l = lrelu(xl2o + xr2o + ef2d)
        ex2l = np.exp((m2l * wp["att2r"][0][None, :]).sum(-1))   # [P]
        numer2 = meta2[0:8] + (xl2o * ex2l[:, None]).T
        denom2 = meta2[8] + ex2l
        o2 = numer2 / denom2[None, :] + wp["bias2e"]
        o2 = elu(o2)
        ylin = wp["Wlin"].T @ o2 + wp["blin"]
        y[b * P : (b + 1) * P] = (1.0 / (1.0 + np.exp(-ylin))).T
    return y


# --------------------------------------------------------------------------
# device program
# --------------------------------------------------------------------------

def build_nc(npc, cpb, n_pad, debug=False):
    nc = bacc.Bacc("TRN2", target_bir_lowering=False)
    npcP = npc * P

    xp_d = nc.dram_tensor("xp", [n_pad, H1], FP, kind="ExternalInput")
    xown_d = nc.dram_tensor("xown", [npcP, H1], FP, kind="ExternalInput")
    rec_d = nc.dram_tensor("rec", [npc, P, cpb * NF], FP, kind="ExternalInput")
    rdt_d = nc.dram_tensor("rdt", [P, npc], FP, kind="ExternalInput")
    wnames = dict(
        W1l=[H1, H1], W1r=[H1, H1], We1a=[EDIM + 1, H1], attB=[P, H1],
        E4=[HEADS, H1], bias1e=[H1, 1], W2l=[H1, C2], W2r=[H1, C2],
        We2a=[EDIM + 1, C2], att2r=[P, C2], bias2e=[C2, 1], ones18=[1, C2],
        Wlin=[C2, C2], blin=[C2, 1],
    )
    wd = {k: nc.dram_tensor(k, sh, FP, kind="ExternalInput")
          for k, sh in wnames.items()}
    y_d = nc.dram_tensor("y", [npcP, OUT], FP, kind="ExternalOutput")
    xl2loc_d = nc.dram_tensor("xl2loc", [npcP, C2], FP)
    if debug:
        xl2dbg_d = nc.dram_tensor("xl2dbg", [npcP, C2], FP, kind="ExternalOutput")
    xr2loc_d = nc.dram_tensor("xr2loc", [npcP, C2], FP)
    xl2full_d = nc.dram_tensor("xl2full", [n_pad, C2], FP, addr_space="Shared")

    PRELU = mybir.ActivationFunctionType.Prelu
    EXP = mybir.ActivationFunctionType.Exp
    RELU = mybir.ActivationFunctionType.Relu
    COPY = mybir.ActivationFunctionType.Copy
    SIGM = mybir.ActivationFunctionType.Sigmoid
    ADD = mybir.AluOpType.add
    MULT = mybir.AluOpType.mult
    MIN = mybir.AluOpType.min
    ISEQ = mybir.AluOpType.is_equal

    from contextlib import ExitStack

    with tile.TileContext(nc) as tc, ExitStack() as stack:
        cp = stack.enter_context(tc.tile_pool(name="consts", bufs=1))
        bp = stack.enter_context(tc.tile_pool(name="big", bufs=2))
        sp = stack.enter_context(tc.tile_pool(name="small", bufs=3))
        pt = stack.enter_context(tc.tile_pool(name="ptp", bufs=2, space="PSUM"))
        pm = stack.enter_context(tc.tile_pool(name="pm", bufs=2, space="PSUM"))
        pa = stack.enter_context(tc.tile_pool(name="pacc", bufs=1, space="PSUM"))

        ident = cp.tile([P, P], FP)
        make_identity(nc, ident[:])
        iota_i = cp.tile([P, P], I32)
        nc.gpsimd.iota(iota_i[:], pattern=[[1, P]], base=0, channel_multiplier=0)
        iota_f = cp.tile([P, P], FP)
        nc.vector.tensor_copy(iota_f[:], iota_i[:])
        alpha02 = cp.tile([P, 1], FP)
        nc.vector.memset(alpha02[:], 0.2)
        w = {}
        for k, sh in wnames.items():
            w[k] = cp.tile(sh, FP, name=f"w_{k}", tag=f"w_{k}")
            nc.sync.dma_start(w[k][:], wd[k][:])
        rdt = cp.tile([P, npc], FP)
        nc.sync.dma_start(rdt[:], rdt_d[:])
        easum_all = cp.tile([EDIM + 1, npc * P], FP)
        ylin_all = cp.tile([C2, npc * P], FP)

        # ---------------- layer 1 ----------------
        for b in range(npc):
            rec = bp.tile([P, cpb * NF], FP, tag="rec")
            nc.sync.dma_start(rec[:], rec_d[b, :, :])
            rec_v = rec[:].rearrange("p (c f) -> p c f", f=NF)
            idx = rec_v[:, :, F_SRC : F_SRC + 1].bitcast(I32)

            xg = bp.tile([P, cpb, H1], FP, tag="xg")
            for c in range(cpb):
                nc.gpsimd.indirect_dma_start(
                    out=xg[:, c, :], out_offset=None, in_=xp_d[:],
                    in_offset=bass.IndirectOffsetOnAxis(
                        ap=rec_v[:, c, F_SRC : F_SRC + 1].bitcast(I32), axis=0))

            xow = bp.tile([P, H1], FP, tag="xow")
            nc.sync.dma_start(xow[:], xown_d[b * P : (b + 1) * P, :])
            xot_ps = pt.tile([P, P], FP, tag="tp")
            nc.tensor.transpose(out=xot_ps[:], in_=xow[:], identity=ident[:])
            xot = bp.tile([P, P], FP, tag="xot")
            nc.scalar.activation(xot[:], xot_ps[:], COPY)
            xr_ps = pt.tile([P, P], FP, tag="tp")
            nc.tensor.matmul(xr_ps[:], lhsT=xot[:], rhs=w["W1r"][:],
                             start=True, stop=True)
            xr = bp.tile([P, H1], FP, tag="xr")
            nc.scalar.activation(xr[:], xr_ps[:], COPY)

            m_all = bp.tile([P, cpb * H1], FP, tag="m_all")
            xl_all = bp.tile([P, cpb * H1], FP, tag="xl_all")
            oh_all = bp.tile([P, cpb, P], FP, tag="oh_all")
            m_all_v = m_all[:].rearrange("p (c f) -> p c f", f=H1)
            xl_all_v = xl_all[:].rearrange("p (c f) -> p c f", f=H1)

            for c in range(cpb):
                xet_ps = pt.tile([P, P], FP, tag="tp")
                nc.tensor.transpose(out=xet_ps[:], in_=xg[:, c, :],
                                    identity=ident[:])
                xet = sp.tile([P, P], FP, tag="xet")
                nc.scalar.activation(xet[:], xet_ps[:], COPY)

                nc.vector.tensor_scalar(
                    out=oh_all[:, c, :], in0=iota_f[:],
                    scalar1=rec_v[:, c, F_DST : F_DST + 1],
                    scalar2=None, op0=ISEQ)
                oht_ps = pt.tile([P, P], FP, tag="tp")
                nc.tensor.transpose(out=oht_ps[:], in_=oh_all[:, c, :],
                                    identity=ident[:])
                oht = sp.tile([P, P], FP, tag="oht")
                nc.vector.tensor_copy(oht[:], oht_ps[:])

                eat_ps = pt.tile([EDIM + 1, P], FP, tag="tp")
                nc.tensor.transpose(out=eat_ps[:],
                                    in_=rec_v[:, c, F_EA : F_EA + 17],
                                    identity=ident[:])
                eat = sp.tile([EDIM + 1, P], FP, tag="eat")
                nc.scalar.activation(eat[:], eat_ps[:], COPY)

                m_ps = pm.tile([P, H1], FP, tag="m")
                nc.tensor.matmul(m_ps[:], lhsT=xet[:], rhs=w["W1l"][:],
                                 start=True, stop=True)
                nc.scalar.activation(xl_all_v[:, c, :], m_ps[:], COPY)
                nc.tensor.matmul(m_ps[:], lhsT=oht[:], rhs=xr[:],
                                 start=False, stop=False, skip_group_check=True)
                nc.tensor.matmul(m_ps[:], lhsT=eat[:], rhs=w["We1a"][:],
                                 start=False, stop=True, skip_group_check=True)
                nc.scalar.activation(m_all_v[:, c, :], m_ps[:], PRELU, alpha=alpha02[:])

            # logits / softmax numerators (block level)
            nc.vector.tensor_tensor(
                out=m_all_v[:], in0=m_all_v[:],
                in1=w["attB"][:].unsqueeze(1).to_broadcast([P, cpb, H1]),
                op=MULT)
            logits = bp.tile([P, cpb * HEADS], FP, tag="logits")
            nc.vector.tensor_reduce(
                out=logits[:].rearrange("p (c h) -> p c h", h=HEADS),
                in_=m_all[:].rearrange("p (c h k) -> p c h k", h=HEADS, k=HC),
                axis=mybir.AxisListType.X, op=ADD)
            nc.scalar.activation(
                rec_v[:, :, F_EX : F_EX + HEADS],
                logits[:].rearrange("p (c h) -> p c h", h=HEADS), EXP)
            nc.vector.tensor_tensor(
                out=xl_all[:].rearrange("p (c h k) -> p c h k", h=HEADS, k=HC),
                in0=xl_all[:].rearrange("p (c h k) -> p c h k", h=HEADS, k=HC),
                in1=rec_v[:, :, F_EX : F_EX + HEADS]
                    .unsqueeze(3).to_broadcast([P, cpb, HEADS, HC]),
                op=MULT)

            numerT_ps = pa.tile([P, P], FP, tag="numerT")
            denom_ps = pa.tile([HEADS, P], FP, tag="denom")
            easum_ps = pa.tile([EDIM + 1, P], FP, tag="easum")
            for c in range(cpb):
                nc.tensor.matmul(numerT_ps[:], lhsT=xl_all_v[:, c, :],
                                 rhs=oh_all[:, c, :],
                                 start=(c == 0), stop=False,
                                 skip_group_check=True)
                nc.tensor.matmul(denom_ps[:], lhsT=rec_v[:, c, F_EX : F_EX + 4],
                                 rhs=oh_all[:, c, :],
                                 start=(c == 0), stop=False,
                                 skip_group_check=True)
                nc.tensor.matmul(easum_ps[:], lhsT=rec_v[:, c, F_EA : F_EA + 17],
                                 rhs=oh_all[:, c, :],
                                 start=(c == 0), stop=(c == cpb - 1),
                                 skip_group_check=True)

            # loop chunk
            nc.scalar.activation(easum_all[:, b * P : (b + 1) * P],
                                 easum_ps[:], COPY)
            efd_ps = pt.tile([P, P], FP, tag="tp")
            nc.tensor.matmul(efd_ps[:],
                             lhsT=easum_all[:, b * P : (b + 1) * P],
                             rhs=w["We1a"][:], start=True, stop=True)
            efd = sp.tile([P, H1], FP, tag="efd")
            nc.vector.tensor_scalar(
                out=efd[:], in0=efd_ps[:], scalar1=rdt[:, b : b + 1],
                scalar2=None, op0=MULT)
            ml_ps = pm.tile([P, H1], FP, tag="m")
            nc.tensor.matmul(ml_ps[:], lhsT=xot[:], rhs=w["W1l"][:],
                             start=True, stop=True)
            xll = sp.tile([P, H1], FP, tag="xll")
            nc.scalar.activation(xll[:], ml_ps[:], COPY)
            nc.tensor.matmul(ml_ps[:], lhsT=ident[:], rhs=xr[:],
                             start=False, stop=False, skip_group_check=True)
            nc.tensor.matmul(ml_ps[:], lhsT=ident[:], rhs=efd[:],
                             start=False, stop=True, skip_group_check=True)
            mloop = sp.tile([P, H1], FP, tag="mloop")
            nc.scalar.activation(mloop[:], ml_ps[:], PRELU, alpha=alpha02[:])
            nc.vector.tensor_tensor(out=mloop[:], in0=mloop[:],
                                    in1=w["attB"][:], op=MULT)
            lgl = sp.tile([P, HEADS], FP, tag="lgl")
            nc.vector.tensor_reduce(
                out=lgl[:],
                in_=mloop[:].rearrange("p (h k) -> p h k", h=HEADS),
                axis=mybir.AxisListType.X, op=ADD)
            exl = sp.tile([P, HEADS], FP, tag="exl")
            nc.scalar.activation(exl[:], lgl[:], EXP)
            nc.tensor.matmul(denom_ps[:], lhsT=exl[:], rhs=ident[:],
                             start=False, stop=True, skip_group_check=True)
            xlwl = sp.tile([P, H1], FP, tag="xlwl")
            nc.vector.tensor_tensor(
                out=xlwl[:].rearrange("p (h k) -> p h k", h=HEADS),
                in0=xll[:].rearrange("p (h k) -> p h k", h=HEADS),
                in1=exl[:].unsqueeze(2).to_broadcast([P, HEADS, HC]),
                op=MULT)
            nc.tensor.matmul(numerT_ps[:], lhsT=xlwl[:], rhs=ident[:],
                             start=False, stop=True, skip_group_check=True)

            # finalize block: hT = elu(numerT/denom + bias1e)
            recip = sp.tile([HEADS, P], FP, tag="recip")
            nc.vector.reciprocal(recip[:], denom_ps[:])
            rfull_ps = pt.tile([P, P], FP, tag="tp")
            nc.tensor.matmul(rfull_ps[:], lhsT=w["E4"][:], rhs=recip[:],
                             start=True, stop=True)
            rfull = sp.tile([P, P], FP, tag="rfull")
            nc.scalar.activation(rfull[:], rfull_ps[:], COPY)
            hT = sp.tile([P, P], FP, tag="hT")
            nc.vector.tensor_tensor(out=hT[:], in0=numerT_ps[:],
                                    in1=rfull[:], op=MULT)
            tmin = sp.tile([P, P], FP, tag="tmin")
            nc.vector.tensor_scalar(out=tmin[:], in0=hT[:],
                                    scalar1=w["bias1e"][:], scalar2=0.0,
                                    op0=ADD, op1=MIN)
            ue = sp.tile([P, P], FP, tag="ue")
            nc.scalar.activation(ue[:], tmin[:], EXP)
            re = sp.tile([P, P], FP, tag="re")
            nc.scalar.activation(re[:], hT[:], RELU, bias=w["bias1e"][:])
            nc.vector.tensor_tensor(out=hT[:], in0=re[:], in1=ue[:], op=ADD)
            nc.vector.tensor_scalar(out=hT[:], in0=hT[:], scalar1=-1.0,
                                    scalar2=None, op0=ADD)

            xl2_ps = pt.tile([P, C2], FP, tag="tp")
            nc.tensor.matmul(xl2_ps[:], lhsT=hT[:], rhs=w["W2l"][:],
                             start=True, stop=True)
            xl2 = sp.tile([P, C2], FP, tag="xl2")
            nc.vector.tensor_copy(xl2[:], xl2_ps[:])
            nc.sync.dma_start(xl2loc_d[b * P : (b + 1) * P, :], xl2[:])
            if debug:
                nc.sync.dma_start(xl2dbg_d[b * P : (b + 1) * P, :], xl2[:])
            xr2_ps = pt.tile([P, C2], FP, tag="tp")
            nc.tensor.matmul(xr2_ps[:], lhsT=hT[:], rhs=w["W2r"][:],
                             start=True, stop=True)
            xr2 = sp.tile([P, C2], FP, tag="xr2")
            nc.vector.tensor_copy(xr2[:], xr2_ps[:])
            nc.sync.dma_start(xr2loc_d[b * P : (b + 1) * P, :], xr2[:])

        # ---------------- exchange ----------------
        nc.gpsimd.collective_compute(
            "AllGather", mybir.AluOpType.bypass,
            replica_groups=[list(range(NCORES))],
            ins=[xl2loc_d[:]], outs=[xl2full_d[:]])

        # ---------------- layer 2 ----------------
        for b in range(npc):
            rec = bp.tile([P, cpb * NF], FP, tag="rec")
            nc.sync.dma_start(rec[:], rec_d[b, :, :])
            rec_v = rec[:].rearrange("p (c f) -> p c f", f=NF)
            idxs = rec_v[:, :, F_SRC : F_SRC + 1].bitcast(I32)
            idxd = rec_v[:, :, F_DLOC : F_DLOC + 1].bitcast(I32)

            xl2g = bp.tile([P, cpb, C2], FP, tag="xl2g")
            xr2g = bp.tile([P, cpb, C2], FP, tag="xr2g")
            for c in range(cpb):
                nc.gpsimd.indirect_dma_start(
                    out=xl2g[:, c, :], out_offset=None, in_=xl2full_d[:],
                    in_offset=bass.IndirectOffsetOnAxis(
                        ap=rec_v[:, c, F_SRC : F_SRC + 1].bitcast(I32), axis=0))
                nc.gpsimd.indirect_dma_start(
                    out=xr2g[:, c, :], out_offset=None, in_=xr2loc_d[:],
                    in_offset=bass.IndirectOffsetOnAxis(
                        ap=rec_v[:, c, F_DLOC : F_DLOC + 1].bitcast(I32), axis=0))

            m2_all = bp.tile([P, cpb * C2], FP, tag="m2_all")
            m2_v = m2_all[:].rearrange("p (c f) -> p c f", f=C2)
            oh_all = bp.tile([P, cpb, P], FP, tag="oh_all")
            for c in range(cpb):
                nc.vector.tensor_scalar(
                    out=oh_all[:, c, :], in0=iota_f[:],
                    scalar1=rec_v[:, c, F_DST : F_DST + 1],
                    scalar2=None, op0=ISEQ)
                eat_ps = pt.tile([EDIM + 1, P], FP, tag="tp")
                nc.tensor.transpose(out=eat_ps[:],
                                    in_=rec_v[:, c, F_EA : F_EA + 17],
                                    identity=ident[:])
                eat = sp.tile([EDIM + 1, P], FP, tag="eat")
                nc.scalar.activation(eat[:], eat_ps[:], COPY)
                m2_ps = pm.tile([P, C2], FP, tag="m")
                nc.tensor.matmul(m2_ps[:], lhsT=eat[:], rhs=w["We2a"][:],
                                 start=True, stop=True)
                t1 = sp.tile([P, C2], FP, tag="t1")
                nc.vector.tensor_tensor(out=t1[:], in0=xl2g[:, c, :],
                                        in1=xr2g[:, c, :], op=ADD)
                nc.vector.tensor_tensor(out=t1[:], in0=t1[:], in1=m2_ps[:],
                                        op=ADD)
                nc.scalar.activation(m2_v[:, c, :], t1[:], PRELU, alpha=alpha02[:])

            nc.vector.tensor_tensor(
                out=m2_v[:], in0=m2_v[:],
                in1=w["att2r"][:].unsqueeze(1).to_broadcast([P, cpb, C2]),
                op=MULT)
            lg2 = bp.tile([P, cpb], FP, tag="lg2")
            nc.vector.tensor_reduce(out=lg2[:], in_=m2_v[:],
                                    axis=mybir.AxisListType.X, op=ADD)
            x9 = bp.tile([P, cpb, C2 + 1], FP, tag="x9")
            nc.scalar.activation(x9[:, :, C2 : C2 + 1],
                                 lg2[:].unsqueeze(2), EXP)
            nc.vector.tensor_tensor(
                out=x9[:, :, 0:C2], in0=xl2g[:],
                in1=x9[:, :, C2 : C2 + 1].to_broadcast([P, cpb, C2]),
                op=MULT)

            numer2_ps = pa.tile([C2, P], FP, tag="numerT")
            denom2_ps = pa.tile([1, P], FP, tag="denom")
            for c in range(cpb):
                nc.tensor.matmul(numer2_ps[:], lhsT=x9[:, c, 0:C2],
                                 rhs=oh_all[:, c, :],
                                 start=(c == 0), stop=False,
                                 skip_group_check=True)
                nc.tensor.matmul(denom2_ps[:], lhsT=x9[:, c, C2 : C2 + 1],
                                 rhs=oh_all[:, c, :],
                                 start=(c == 0), stop=False,
                                 skip_group_check=True)

            # loop chunk
            ef2_ps = pt.tile([P, C2], FP, tag="tp")
            nc.tensor.matmul(ef2_ps[:],
                             lhsT=easum_all[:, b * P : (b + 1) * P],
                             rhs=w["We2a"][:], start=True, stop=True)
            xl2o = sp.tile([P, C2], FP, tag="xl2o")
            nc.sync.dma_start(xl2o[:], xl2loc_d[b * P : (b + 1) * P, :])
            xr2o = sp.tile([P, C2], FP, tag="xr2o")
            nc.sync.dma_start(xr2o[:], xr2loc_d[b * P : (b + 1) * P, :])
            m2l = sp.tile([P, C2], FP, tag="m2l")
            nc.vector.tensor_scalar(out=m2l[:], in0=ef2_ps[:],
                                    scalar1=rdt[:, b : b + 1], scalar2=None,
                                    op0=MULT)
            nc.vector.tensor_tensor(out=m2l[:], in0=m2l[:], in1=xl2o[:], op=ADD)
            nc.vector.tensor_tensor(out=m2l[:], in0=m2l[:], in1=xr2o[:], op=ADD)
            nc.scalar.activation(m2l[:], m2l[:], PRELU, alpha=alpha02[:])
            nc.vector.tensor_tensor(out=m2l[:], in0=m2l[:], in1=w["att2r"][:],
                                    op=MULT)
            x9l = sp.tile([P, C2 + 1], FP, tag="x9l")
            nc.vector.tensor_reduce(out=x9l[:, C2 : C2 + 1], in_=m2l[:],
                                    axis=mybir.AxisListType.X, op=ADD)
            nc.scalar.activation(x9l[:, C2 : C2 + 1], x9l[:, C2 : C2 + 1], EXP)
            nc.vector.tensor_scalar(out=x9l[:, 0:C2], in0=xl2o[:],
                                    scalar1=x9l[:, C2 : C2 + 1], scalar2=None,
                                    op0=MULT)
            nc.tensor.matmul(numer2_ps[:], lhsT=x9l[:, 0:C2], rhs=ident[:],
                             start=False, stop=True, skip_group_check=True)
            nc.tensor.matmul(denom2_ps[:], lhsT=x9l[:, C2 : C2 + 1],
                             rhs=ident[:],
                             start=False, stop=True, skip_group_check=True)

            # finalize
            rc2 = sp.tile([1, P], FP, tag="rc2")
            nc.vector.reciprocal(rc2[:], denom2_ps[:])
            r2f_ps = pt.tile([C2, P], FP, tag="tp")
            nc.tensor.matmul(r2f_ps[:], lhsT=w["ones18"][:], rhs=rc2[:],
                             start=True, stop=True)
            r2f = sp.tile([C2, P], FP, tag="r2f")
            nc.scalar.activation(r2f[:], r2f_ps[:], COPY)
            o2 = sp.tile([C2, P], FP, tag="o2")
            nc.vector.tensor_tensor(out=o2[:], in0=numer2_ps[:],
                                    in1=r2f[:], op=MULT)
            t2m = sp.tile([C2, P], FP, tag="t2m")
            nc.vector.tensor_scalar(out=t2m[:], in0=o2[:],
                                    scalar1=w["bias2e"][:], scalar2=0.0,
                                    op0=ADD, op1=MIN)
            u2 = sp.tile([C2, P], FP, tag="u2")
            nc.scalar.activation(u2[:], t2m[:], EXP)
            r2 = sp.tile([C2, P], FP, tag="r2")
            nc.scalar.activation(r2[:], o2[:], RELU, bias=w["bias2e"][:])
            nc.vector.tensor_tensor(out=o2[:], in0=r2[:], in1=u2[:], op=ADD)
            nc.vector.tensor_scalar(out=o2[:], in0=o2[:], scalar1=-1.0,
                                    scalar2=None, op0=ADD)
            ylin_ps = pt.tile([C2, P], FP, tag="tp")
            nc.tensor.matmul(ylin_ps[:], lhsT=w["Wlin"][:], rhs=o2[:],
                             start=True, stop=True)
            nc.scalar.activation(ylin_all[:, b * P : (b + 1) * P],
                                 ylin_ps[:], COPY)

        # ---------------- output ----------------
        ysig = cp.tile([C2, npc * P], FP)
        nc.scalar.activation(ysig[:], ylin_all[:], SIGM, bias=w["blin"][:])
        for b in range(npc):
            yt_ps = pt.tile([P, C2], FP, tag="tp")
            nc.tensor.transpose(out=yt_ps[:],
                                in_=ysig[:, b * P : (b + 1) * P],
                                identity=ident[0:C2, 0:C2])
            yt = sp.tile([P, C2], FP, tag="yt")
            nc.vector.tensor_copy(yt[:], yt_ps[:])
            nc.sync.dma_start(y_d[b * P : (b + 1) * P, :], yt[:])
    return nc


# --------------------------------------------------------------------------
# runners
# --------------------------------------------------------------------------

def make_in_maps(pp, wp, npc):
    n_pad, nb = pp["n_pad"], pp["nb"]
    xp, rec, rdt = pp["xp"], pp["rec"], pp["rdt"]
    in_maps = []
    for c in range(NCORES):
        m = dict(
            xp=xp,
            xown=xp[c * npc * P : (c + 1) * npc * P],
            rec=np.ascontiguousarray(
                rec[c * npc : (c + 1) * npc].reshape(npc, P, -1)),
            rdt=np.ascontiguousarray(rdt[:, c * npc : (c + 1) * npc]),
        )
        m.update(wp)
        in_maps.append(m)
    return in_maps


def run_graph(inputs, npc, backend="hw", trace=False, debug=False):
    """Full pipeline: prep on host, run on 8 cores, unpermute."""
    x = np.asarray(inputs["x"], np.float32)
    n = x.shape[0]
    pp = prep(x, inputs["edge_index"], inputs["edge_attr"], npc)
    wp = prep_weights(inputs)
    nc = build_nc(npc, pp["cpb"], pp["n_pad"], debug=debug)
    nc.compile()
    in_maps = make_in_maps(pp, wp, npc)
    info = {}
    if backend == "sim":
        from concourse.bass_interp import MultiCoreSim
        sim = MultiCoreSim(nc, num_cores=NCORES,
                           require_finite=False, require_nnan=False)
        for c in range(NCORES):
            core = sim.cores[c]
            for k, v in in_maps[c].items():
                core.tensor(k)[:] = v
        sim.simulate()
        outs = [sim.cores[c].tensor("y") for c in range(NCORES)]
    else:
        from concourse.bass_utils import run_bass_kernel_spmd
        res = run_bass_kernel_spmd(nc, in_maps, list(range(NCORES)),
                                   trace=trace)
        outs = [res.results[c]["y"] for c in range(NCORES)]
        if debug:
            info["xl2loc"] = np.concatenate(
                [res.results[c]["xl2dbg"] for c in range(NCORES)], axis=0)
        info["exec_time_ns"] = res.exec_time_ns
        info["profile_json"] = getattr(res, "profile_json", None)
    yp = np.concatenate(outs, axis=0)         # [n_pad, OUT] permuted order
    y = yp[pp["permpos"][:n]]
    return np.ascontiguousarray(y), info


def kernel(**inputs):
    y, _ = run_graph(inputs, npc=49, backend="hw")
    return y


# revision 19
# speedup vs baseline: 1.0292x; 1.0292x over previous
"""GATv2 (2-layer) + linear head GNN kernel for Trainium2, 8 NeuronCores.

Strategy: nodes are permuted into degree-balanced blocks of 128; blocks are
sharded contiguously across 8 cores. Each core processes all edges whose
destination lies in its blocks (edges grouped by destination block on the
host). Segment softmax and scatter-add are block-local: per 128-edge chunk a
one-hot (edge x node) matrix is built on-device with an iota/is_equal compare
and used as a matmul operand, so PSUM accumulates the per-node exp-sums and
weighted feature sums. Self loops are handled by a per-block "loop chunk"
whose edge features are the on-device computed mean of incoming edge_attr.
Layer-2 source projections are exchanged with a single AllGather.
"""
import sys

sys.path.insert(0, "/opt/trn_rl_repo")

import numpy as np
import concourse.bass as bass
import concourse.mybir as mybir
import concourse.tile as tile
from concourse import bacc
from concourse.masks import make_identity

P = 128
HEADS = 4
HC = 32          # channels per head, layer 1
H1 = HEADS * HC  # 128
C2 = 8
EDIM = 16
OUT = 8
NCORES = 8
NF = 24          # rec fields: [ex*4 | ea*16 | one | srcp_i32 | dstf | dstloc_i32]
F_EX = 0
F_EA = 4
F_ONE = 20
F_SRC = 21
F_DST = 22
F_DLOC = 23
PAD_DST = 999.0

FP = mybir.dt.float32
I32 = mybir.dt.int32


# --------------------------------------------------------------------------
# host-side preprocessing
# --------------------------------------------------------------------------

def balanced_blocks(deg, n_pad):
    """Assign each node (0..n_pad-1) to a block of exactly P nodes so block
    degree sums are balanced. Returns permpos[node] = block*P + slot."""
    import heapq

    nb = n_pad // P
    order = np.argsort(-deg, kind="stable")
    heap = [(0, b) for b in range(nb)]
    heapq.heapify(heap)
    counts = np.zeros(nb, np.int64)
    permpos = np.empty(n_pad, np.int64)
    slot_of = np.zeros(nb, np.int64)
    for node in order:
        while True:
            s, b = heapq.heappop(heap)
            if counts[b] < P:
                break
        permpos[node] = b * P + slot_of[b]
        slot_of[b] += 1
        counts[b] += 1
        if counts[b] < P:
            heapq.heappush(heap, (s + int(deg[node]), b))
    return permpos


def prep(x, edge_index, edge_attr, npc, cpb=None):
    """Build permuted/padded inputs and per-core edge records."""
    n, din = x.shape
    e = edge_attr.shape[0]
    n_pad = NCORES * npc * P
    nb = n_pad // P
    src = np.asarray(edge_index[0], np.int64)
    dst = np.asarray(edge_index[1], np.int64)

    deg = np.bincount(dst, minlength=n_pad).astype(np.int64)
    permpos = balanced_blocks(deg, n_pad)

    orderv = np.empty(n_pad, np.int64)
    orderv[permpos] = np.arange(n_pad)
    xp = np.zeros((n_pad, din), np.float32)
    xp[permpos[:n]] = np.asarray(x, np.float32)

    blk = permpos // P        # block of each node
    slot = permpos % P

    eb = blk[dst]             # block of each edge
    # order edges by block
    eorder = np.argsort(eb, kind="stable")
    eb_s = eb[eorder]
    counts = np.bincount(eb_s, minlength=nb)
    need = int(np.ceil(counts.max() / P))
    if cpb is None:
        cpb = need
    assert cpb >= need, (cpb, need)

    rec = np.zeros((nb, P, cpb, NF), np.float32)
    rec[:, :, :, F_DST] = PAD_DST
    # positions within block
    starts = np.zeros(nb + 1, np.int64)
    np.cumsum(counts, out=starts[1:])
    pos_in_blk = np.arange(e) - starts[eb_s]
    cc = pos_in_blk // P
    pp = pos_in_blk % P
    es, ed = src[eorder], dst[eorder]
    rec[eb_s, pp, cc, F_SRC] = permpos[es].astype(np.int32).view(np.float32)
    rec[eb_s, pp, cc, F_DST] = slot[ed].astype(np.float32)
    rec[eb_s, pp, cc, F_DLOC] = (
        ((eb_s % npc) * P + slot[ed]).astype(np.int32).view(np.float32)
    )
    rec[eb_s, pp, cc, F_EA : F_EA + EDIM] = np.asarray(edge_attr, np.float32)[eorder]
    rec[eb_s, pp, cc, F_ONE] = 1.0
    # pad slots: srcp/dstloc default 0 (valid), dstf=999 disables them

    rd = 1.0 / np.maximum(deg.astype(np.float32), 1.0)
    rdp = np.empty(n_pad, np.float32)
    rdp[permpos] = rd
    # rdt per core: [P, npc]
    rdt = rdp.reshape(nb, P).transpose(1, 0)  # [P, nb]

    # host-transposed [ea|one] per block: [nb, 17, cpb*P]
    eat = np.ascontiguousarray(
        rec[:, :, :, F_EA : F_EA + 17].transpose(0, 3, 2, 1)
    ).reshape(nb, EDIM + 1, cpb * P)

    return dict(
        xp=xp, rec=rec, rdt=rdt, eat=eat, permpos=permpos, orderv=orderv,
        n_pad=n_pad, nb=nb, cpb=cpb,
    )


def prep_weights(w):
    """Host-side packing of model weights."""
    W1l = np.asarray(w["W1l"], np.float32)
    W1r = np.asarray(w["W1r"], np.float32)
    We1a = np.concatenate(
        [np.asarray(w["We1"], np.float32),
         (np.asarray(w["b1l"]) + np.asarray(w["b1r"])).astype(np.float32)[None, :]],
        axis=0,
    )  # [17, H1]
    att1 = np.asarray(w["att1"], np.float32)        # [HEADS, HC]
    attB = np.zeros((P, H1), np.float32)
    attB[:] = att1.reshape(-1)[None, :]             # [P, 128] replicated rows
    E4 = np.zeros((HEADS, H1), np.float32)
    for h in range(HEADS):
        E4[h, h * HC : (h + 1) * HC] = 1.0
    bias1e = (np.asarray(w["bias1"]) + np.asarray(w["b1l"])).astype(np.float32)[:, None]

    W2l = np.asarray(w["W2l"], np.float32)
    W2r = np.asarray(w["W2r"], np.float32)
    We2a = np.concatenate(
        [np.asarray(w["We2"], np.float32),
         (np.asarray(w["b2l"]) + np.asarray(w["b2r"])).astype(np.float32)[None, :]],
        axis=0,
    )  # [17, C2]
    att2r = np.zeros((P, C2), np.float32)
    att2r[:] = np.asarray(w["att2"], np.float32).reshape(-1)[None, :]
    bias2e = (np.asarray(w["bias2"]) + np.asarray(w["b2l"])).astype(np.float32)[:, None]
    ones18 = np.ones((1, C2), np.float32)
    Wlin = np.asarray(w["Wlin"], np.float32)
    blin = np.asarray(w["blin"], np.float32)[:, None]
    return dict(
        W1l=W1l, W1r=W1r, We1a=We1a, attB=attB, E4=E4, bias1e=bias1e,
        W2l=W2l, W2r=W2r, We2a=We2a, att2r=att2r, bias2e=bias2e,
        ones18=ones18, Wlin=Wlin, blin=blin,
    )


# --------------------------------------------------------------------------
# numpy emulator of the device algorithm (for debugging, not used by kernel)
# --------------------------------------------------------------------------

def np_forward(pp, wp, npc):
    xp, rec, rdt = pp["xp"], pp["rec"], pp["rdt"]
    nb, _, cpb, _ = rec.shape
    n_pad = pp["n_pad"]
    rec = rec.copy()

    def lrelu(v):
        return np.where(v > 0, v, 0.2 * v)

    def elu(v):
        return np.where(v > 0, v, np.exp(np.minimum(v, 0)) - 1.0)

    xl2loc = np.zeros((n_pad, C2), np.float32)
    xr2loc = np.zeros((n_pad, C2), np.float32)
    easum_all = np.zeros((nb, 17, P), np.float32)

    # layer 1 per block
    hT_all = np.zeros((nb, H1, P), np.float32)
    for b in range(nb):
        r = rec[b]                      # [P, cpb, NF]
        srcp = r[:, :, F_SRC].view(np.int32)
        dstf = r[:, :, F_DST]
        oh = (dstf[:, :, None] == np.arange(P)[None, None, :]).astype(np.float32)
        xg = xp[srcp]                   # [P, cpb, 128]
        xown = xp[b * P : (b + 1) * P]  # [P, 128]
        xr = xown @ wp["W1r"]
        xl = xg @ wp["W1l"]             # [P, cpb, 128]
        eaa = r[:, :, F_EA : F_EA + 17]
        m = lrelu(xl + np.einsum("pcn,nf->pcf", oh, xr)
                  + eaa @ wp["We1a"])
        logits = (m * wp["attB"][0][None, None, :]).reshape(P, cpb, HEADS, HC).sum(-1)
        ex = np.exp(logits)             # [P, cpb, 4]
        r[:, :, F_EX : F_EX + 4] = ex
        xlw = xl * np.repeat(ex, HC, axis=2)
        numerT = np.einsum("pcf,pcn->fn", xlw, oh)
        meta = np.einsum("pcj,pcn->jn", r[:, :, 0:21], oh)  # [21, P]
        easum = meta[4:21]              # [17, P] (row 16 = deg)
        easum_all[b] = easum
        # loop chunk
        efd = (easum.T @ wp["We1a"]) * rdt[:, b][:, None]
        mloop = lrelu(xown @ wp["W1l"] + xr + efd)
        exl = np.exp(
            (mloop * wp["attB"][0][None, :]).reshape(P, HEADS, HC).sum(-1))
        denom = meta[0:4] + exl.T       # [4, P]
        numerT = numerT + ((xown @ wp["W1l"]) * np.repeat(exl, HC, 1)).T
        rfull = np.repeat(1.0 / denom, HC, axis=0)  # [128, P]
        hT = numerT * rfull + wp["bias1e"]
        hT = elu(hT)
        xl2loc[b * P : (b + 1) * P] = hT.T @ wp["W2l"]
        xr2loc[b * P : (b + 1) * P] = hT.T @ wp["W2r"]
        hT_all[b] = hT

    # layer 2 per block
    y = np.zeros((n_pad, OUT), np.float32)
    for b in range(nb):
        core = b // npc
        r = rec[b]
        srcp = r[:, :, F_SRC].view(np.int32)
        dloc = r[:, :, F_DLOC].view(np.int32)
        dstf = r[:, :, F_DST]
        oh = (dstf[:, :, None] == np.arange(P)[None, None, :]).astype(np.float32)
        xl2g = xl2loc[srcp]             # [P, cpb, 8]
        xr2g = xr2loc[core * npc * P + dloc]
        eaa = r[:, :, F_EA : F_EA + 17]
        m2 = lrelu(xl2g + xr2g + eaa @ wp["We2a"])
        logits2 = (m2 * wp["att2r"][0][None, None, :]).sum(-1)   # [P, cpb]
        ex2 = np.exp(logits2)
        xl2w = xl2g * ex2[:, :, None]
        meta2 = np.einsum(
            "pcj,pcn->jn",
            np.concatenate([xl2w, ex2[:, :, None]], axis=2), oh)  # [9, P]
        # loop
        xl2o = xl2loc[b * P : (b + 1) * P]
        xr2o = xr2loc[b * P : (b + 1) * P]
        ef2d = (easum_all[b].T @ wp["We2a"]) * rdt[:, b][:, None]
        m2l = lrelu(xl2o + xr2o + ef2d)
        ex2l = np.exp((m2l * wp["att2r"][0][None, :]).sum(-1))   # [P]
        numer2 = meta2[0:8] + (xl2o * ex2l[:, None]).T
        denom2 = meta2[8] + ex2l
        o2 = numer2 / denom2[None, :] + wp["bias2e"]
        o2 = elu(o2)
        ylin = wp["Wlin"].T @ o2 + wp["blin"]
        y[b * P : (b + 1) * P] = (1.0 / (1.0 + np.exp(-ylin))).T
    return y


# --------------------------------------------------------------------------
# device program
# --------------------------------------------------------------------------

def build_nc(npc, cpb, n_pad, debug=False):
    nc = bacc.Bacc("TRN2", target_bir_lowering=False)
    npcP = npc * P

    xp_d = nc.dram_tensor("xp", [n_pad, H1], FP, kind="ExternalInput")
    xown_d = nc.dram_tensor("xown", [npcP, H1], FP, kind="ExternalInput")
    rec_d = nc.dram_tensor("rec", [npc, P, cpb * NF], FP, kind="ExternalInput")
    rdt_d = nc.dram_tensor("rdt", [P, npc], FP, kind="ExternalInput")
    eat_d = nc.dram_tensor("eat", [npc, EDIM + 1, cpb * P], FP,
                           kind="ExternalInput")
    wnames = dict(
        W1l=[H1, H1], W1r=[H1, H1], We1a=[EDIM + 1, H1], attB=[P, H1],
        E4=[HEADS, H1], bias1e=[H1, 1], W2l=[H1, C2], W2r=[H1, C2],
        We2a=[EDIM + 1, C2], att2r=[P, C2], bias2e=[C2, 1], ones18=[1, C2],
        Wlin=[C2, C2], blin=[C2, 1],
    )
    wd = {k: nc.dram_tensor(k, sh, FP, kind="ExternalInput")
          for k, sh in wnames.items()}
    y_d = nc.dram_tensor("y", [npcP, OUT], FP, kind="ExternalOutput")
    xl2loc_d = nc.dram_tensor("xl2loc", [npcP, C2], FP)
    if debug:
        xl2dbg_d = nc.dram_tensor("xl2dbg", [npcP, C2], FP, kind="ExternalOutput")
    xr2loc_d = nc.dram_tensor("xr2loc", [npcP, C2], FP)
    xl2full_d = nc.dram_tensor("xl2full", [n_pad, C2], FP, addr_space="Shared")

    PRELU = mybir.ActivationFunctionType.Prelu
    EXP = mybir.ActivationFunctionType.Exp
    RELU = mybir.ActivationFunctionType.Relu
    COPY = mybir.ActivationFunctionType.Copy
    SIGM = mybir.ActivationFunctionType.Sigmoid
    ADD = mybir.AluOpType.add
    MULT = mybir.AluOpType.mult
    MIN = mybir.AluOpType.min
    ISEQ = mybir.AluOpType.is_equal

    from contextlib import ExitStack

    with tile.TileContext(nc) as tc, ExitStack() as stack:
        cp = stack.enter_context(tc.tile_pool(name="consts", bufs=1))
        bp = stack.enter_context(tc.tile_pool(name="big", bufs=2))
        sp = stack.enter_context(tc.tile_pool(name="small", bufs=3))
        pt = stack.enter_context(tc.tile_pool(name="ptp", bufs=2, space="PSUM"))
        pm = stack.enter_context(tc.tile_pool(name="pm", bufs=2, space="PSUM"))
        pa = stack.enter_context(tc.tile_pool(name="pacc", bufs=1, space="PSUM"))

        ident = cp.tile([P, P], FP)
        make_identity(nc, ident[:])
        iota_i = cp.tile([P, P], I32)
        nc.gpsimd.iota(iota_i[:], pattern=[[1, P]], base=0, channel_multiplier=0)
        iota_f = cp.tile([P, P], FP)
        nc.vector.tensor_copy(iota_f[:], iota_i[:])
        alpha02 = cp.tile([P, 1], FP)
        nc.vector.memset(alpha02[:], 0.2)
        w = {}
        for k, sh in wnames.items():
            w[k] = cp.tile(sh, FP, name=f"w_{k}", tag=f"w_{k}")
            nc.sync.dma_start(w[k][:], wd[k][:])
        rdt = cp.tile([P, npc], FP)
        nc.sync.dma_start(rdt[:], rdt_d[:])
        easum_all = cp.tile([EDIM + 1, npc * P], FP)
        ylin_all = cp.tile([C2, npc * P], FP)

        # ---------------- layer 1 ----------------
        for b in range(npc):
            rec = bp.tile([P, cpb * NF], FP, tag="rec")
            nc.sync.dma_start(rec[:], rec_d[b, :, :])
            rec_v = rec[:].rearrange("p (c f) -> p c f", f=NF)
            idx = rec_v[:, :, F_SRC : F_SRC + 1].bitcast(I32)

            xg = bp.tile([P, cpb, H1], FP, tag="xg")
            for c in range(cpb):
                nc.gpsimd.indirect_dma_start(
                    out=xg[:, c, :], out_offset=None, in_=xp_d[:],
                    in_offset=bass.IndirectOffsetOnAxis(
                        ap=rec_v[:, c, F_SRC : F_SRC + 1].bitcast(I32), axis=0))
            eat_t = bp.tile([EDIM + 1, cpb * P], FP, tag="eat_all")
            nc.sync.dma_start(eat_t[:], eat_d[b, :, :])

            xow = bp.tile([P, H1], FP, tag="xow")
            nc.sync.dma_start(xow[:], xown_d[b * P : (b + 1) * P, :])
            xot_ps = pt.tile([P, P], FP, tag="tp")
            nc.tensor.transpose(out=xot_ps[:], in_=xow[:], identity=ident[:])
            xot = bp.tile([P, P], FP, tag="xot")
            nc.scalar.activation(xot[:], xot_ps[:], COPY)
            xr_ps = pt.tile([P, P], FP, tag="tp")
            nc.tensor.matmul(xr_ps[:], lhsT=xot[:], rhs=w["W1r"][:],
                             start=True, stop=True)
            xr = bp.tile([P, H1], FP, tag="xr")
            nc.scalar.activation(xr[:], xr_ps[:], COPY)

            m_all = bp.tile([P, cpb * H1], FP, tag="m_all")
            xl_all = bp.tile([P, cpb * H1], FP, tag="xl_all")
            oh_all = bp.tile([P, cpb, P], FP, tag="oh_all")
            m_all_v = m_all[:].rearrange("p (c f) -> p c f", f=H1)
            xl_all_v = xl_all[:].rearrange("p (c f) -> p c f", f=H1)

            for c in range(cpb):
                xet_ps = pt.tile([P, P], FP, tag="tp")
                nc.tensor.transpose(out=xet_ps[:], in_=xg[:, c, :],
                                    identity=ident[:])
                xet = sp.tile([P, P], FP, tag="xet")
                nc.scalar.activation(xet[:], xet_ps[:], COPY)

                nc.vector.tensor_scalar(
                    out=oh_all[:, c, :], in0=iota_f[:],
                    scalar1=rec_v[:, c, F_DST : F_DST + 1],
                    scalar2=None, op0=ISEQ)
                oht_ps = pt.tile([P, P], FP, tag="tp")
                nc.tensor.transpose(out=oht_ps[:], in_=oh_all[:, c, :],
                                    identity=ident[:])
                oht = sp.tile([P, P], FP, tag="oht")
                nc.vector.tensor_copy(oht[:], oht_ps[:])

                m_ps = pm.tile([P, H1], FP, tag="m")
                nc.tensor.matmul(m_ps[:], lhsT=xet[:], rhs=w["W1l"][:],
                                 start=True, stop=True)
                nc.scalar.activation(xl_all_v[:, c, :], m_ps[:], COPY)
                nc.tensor.matmul(m_ps[:], lhsT=oht[:], rhs=xr[:],
                                 start=False, stop=False, skip_group_check=True)
                nc.tensor.matmul(m_ps[:], lhsT=eat_t[:, c * P : (c + 1) * P],
                                 rhs=w["We1a"][:],
                                 start=False, stop=True, skip_group_check=True)
                nc.scalar.activation(m_all_v[:, c, :], m_ps[:], PRELU, alpha=alpha02[:])

            # logits / softmax numerators (block level)
            nc.vector.tensor_tensor(
                out=m_all_v[:], in0=m_all_v[:],
                in1=w["attB"][:].unsqueeze(1).to_broadcast([P, cpb, H1]),
                op=MULT)
            logits = bp.tile([P, cpb * HEADS], FP, tag="logits")
            nc.vector.tensor_reduce(
                out=logits[:].rearrange("p (c h) -> p c h", h=HEADS),
                in_=m_all[:].rearrange("p (c h k) -> p c h k", h=HEADS, k=HC),
                axis=mybir.AxisListType.X, op=ADD)
            nc.scalar.activation(
                rec_v[:, :, F_EX : F_EX + HEADS],
                logits[:].rearrange("p (c h) -> p c h", h=HEADS), EXP)
            nc.vector.tensor_tensor(
                out=xl_all[:].rearrange("p (c h k) -> p c h k", h=HEADS, k=HC),
                in0=xl_all[:].rearrange("p (c h k) -> p c h k", h=HEADS, k=HC),
                in1=rec_v[:, :, F_EX : F_EX + HEADS]
                    .unsqueeze(3).to_broadcast([P, cpb, HEADS, HC]),
                op=MULT)

            numerT_ps = pa.tile([P, P], FP, tag="numerT")
            denom_ps = pa.tile([HEADS, P], FP, tag="denom")
            easum_ps = pa.tile([EDIM + 1, P], FP, tag="easum")
            for c in range(cpb):
                nc.tensor.matmul(numerT_ps[:], lhsT=xl_all_v[:, c, :],
                                 rhs=oh_all[:, c, :],
                                 start=(c == 0), stop=False,
                                 skip_group_check=True)
                nc.tensor.matmul(denom_ps[:], lhsT=rec_v[:, c, F_EX : F_EX + 4],
                                 rhs=oh_all[:, c, :],
                                 start=(c == 0), stop=False,
                                 skip_group_check=True)
                nc.tensor.matmul(easum_ps[:], lhsT=rec_v[:, c, F_EA : F_EA + 17],
                                 rhs=oh_all[:, c, :],
                                 start=(c == 0), stop=(c == cpb - 1),
                                 skip_group_check=True)

            # loop chunk
            nc.scalar.activation(easum_all[:, b * P : (b + 1) * P],
                                 easum_ps[:], COPY)
            efd_ps = pt.tile([P, P], FP, tag="tp")
            nc.tensor.matmul(efd_ps[:],
                             lhsT=easum_all[:, b * P : (b + 1) * P],
                             rhs=w["We1a"][:], start=True, stop=True)
            efd = sp.tile([P, H1], FP, tag="efd")
            nc.vector.tensor_scalar(
                out=efd[:], in0=efd_ps[:], scalar1=rdt[:, b : b + 1],
                scalar2=None, op0=MULT)
            ml_ps = pm.tile([P, H1], FP, tag="m")
            nc.tensor.matmul(ml_ps[:], lhsT=xot[:], rhs=w["W1l"][:],
                             start=True, stop=True)
            xll = sp.tile([P, H1], FP, tag="xll")
            nc.scalar.activation(xll[:], ml_ps[:], COPY)
            nc.tensor.matmul(ml_ps[:], lhsT=ident[:], rhs=xr[:],
                             start=False, stop=False, skip_group_check=True)
            nc.tensor.matmul(ml_ps[:], lhsT=ident[:], rhs=efd[:],
                             start=False, stop=True, skip_group_check=True)
            mloop = sp.tile([P, H1], FP, tag="mloop")
            nc.scalar.activation(mloop[:], ml_ps[:], PRELU, alpha=alpha02[:])
            nc.vector.tensor_tensor(out=mloop[:], in0=mloop[:],
                                    in1=w["attB"][:], op=MULT)
            lgl = sp.tile([P, HEADS], FP, tag="lgl")
            nc.vector.tensor_reduce(
                out=lgl[:],
                in_=mloop[:].rearrange("p (h k) -> p h k", h=HEADS),
                axis=mybir.AxisListType.X, op=ADD)
            exl = sp.tile([P, HEADS], FP, tag="exl")
            nc.scalar.activation(exl[:], lgl[:], EXP)
            nc.tensor.matmul(denom_ps[:], lhsT=exl[:], rhs=ident[:],
                             start=False, stop=True, skip_group_check=True)
            xlwl = sp.tile([P, H1], FP, tag="xlwl")
            nc.vector.tensor_tensor(
                out=xlwl[:].rearrange("p (h k) -> p h k", h=HEADS),
                in0=xll[:].rearrange("p (h k) -> p h k", h=HEADS),
                in1=exl[:].unsqueeze(2).to_broadcast([P, HEADS, HC]),
                op=MULT)
            nc.tensor.matmul(numerT_ps[:], lhsT=xlwl[:], rhs=ident[:],
                             start=False, stop=True, skip_group_check=True)

            # finalize block: hT = elu(numerT/denom + bias1e)
            recip = sp.tile([HEADS, P], FP, tag="recip")
            nc.vector.reciprocal(recip[:], denom_ps[:])
            rfull_ps = pt.tile([P, P], FP, tag="tp")
            nc.tensor.matmul(rfull_ps[:], lhsT=w["E4"][:], rhs=recip[:],
                             start=True, stop=True)
            rfull = sp.tile([P, P], FP, tag="rfull")
            nc.scalar.activation(rfull[:], rfull_ps[:], COPY)
            hT = sp.tile([P, P], FP, tag="hT")
            nc.vector.tensor_tensor(out=hT[:], in0=numerT_ps[:],
                                    in1=rfull[:], op=MULT)
            tmin = sp.tile([P, P], FP, tag="tmin")
            nc.vector.tensor_scalar(out=tmin[:], in0=hT[:],
                                    scalar1=w["bias1e"][:], scalar2=0.0,
                                    op0=ADD, op1=MIN)
            ue = sp.tile([P, P], FP, tag="ue")
            nc.scalar.activation(ue[:], tmin[:], EXP)
            re = sp.tile([P, P], FP, tag="re")
            nc.scalar.activation(re[:], hT[:], RELU, bias=w["bias1e"][:])
            nc.vector.tensor_tensor(out=hT[:], in0=re[:], in1=ue[:], op=ADD)
            nc.vector.tensor_scalar(out=hT[:], in0=hT[:], scalar1=-1.0,
                                    scalar2=None, op0=ADD)

            xl2_ps = pt.tile([P, C2], FP, tag="tp")
            nc.tensor.matmul(xl2_ps[:], lhsT=hT[:], rhs=w["W2l"][:],
                             start=True, stop=True)
            xl2 = sp.tile([P, C2], FP, tag="xl2")
            nc.vector.tensor_copy(xl2[:], xl2_ps[:])
            nc.sync.dma_start(xl2loc_d[b * P : (b + 1) * P, :], xl2[:])
            if debug:
                nc.sync.dma_start(xl2dbg_d[b * P : (b + 1) * P, :], xl2[:])
            xr2_ps = pt.tile([P, C2], FP, tag="tp")
            nc.tensor.matmul(xr2_ps[:], lhsT=hT[:], rhs=w["W2r"][:],
                             start=True, stop=True)
            xr2 = sp.tile([P, C2], FP, tag="xr2")
            nc.vector.tensor_copy(xr2[:], xr2_ps[:])
            nc.sync.dma_start(xr2loc_d[b * P : (b + 1) * P, :], xr2[:])

        # ---------------- exchange ----------------
        nc.gpsimd.collective_compute(
            "AllGather", mybir.AluOpType.bypass,
            replica_groups=[list(range(NCORES))],
            ins=[xl2loc_d[:]], outs=[xl2full_d[:]])

        # ---------------- layer 2 ----------------
        for b in range(npc):
            rec = bp.tile([P, cpb * NF], FP, tag="rec")
            nc.sync.dma_start(rec[:], rec_d[b, :, :])
            rec_v = rec[:].rearrange("p (c f) -> p c f", f=NF)
            idxs = rec_v[:, :, F_SRC : F_SRC + 1].bitcast(I32)
            idxd = rec_v[:, :, F_DLOC : F_DLOC + 1].bitcast(I32)

            xl2g = bp.tile([P, cpb, C2], FP, tag="xl2g")
            xr2g = bp.tile([P, cpb, C2], FP, tag="xr2g")
            for c in range(cpb):
                nc.gpsimd.indirect_dma_start(
                    out=xl2g[:, c, :], out_offset=None, in_=xl2full_d[:],
                    in_offset=bass.IndirectOffsetOnAxis(
                        ap=rec_v[:, c, F_SRC : F_SRC + 1].bitcast(I32), axis=0))
                nc.gpsimd.indirect_dma_start(
                    out=xr2g[:, c, :], out_offset=None, in_=xr2loc_d[:],
                    in_offset=bass.IndirectOffsetOnAxis(
                        ap=rec_v[:, c, F_DLOC : F_DLOC + 1].bitcast(I32), axis=0))

            eat_t = bp.tile([EDIM + 1, cpb * P], FP, tag="eat_all")
            nc.sync.dma_start(eat_t[:], eat_d[b, :, :])
            m2_all = bp.tile([P, cpb * C2], FP, tag="m2_all")
            m2_v = m2_all[:].rearrange("p (c f) -> p c f", f=C2)
            oh_all = bp.tile([P, cpb, P], FP, tag="oh_all")
            for c in range(cpb):
                nc.vector.tensor_scalar(
                    out=oh_all[:, c, :], in0=iota_f[:],
                    scalar1=rec_v[:, c, F_DST : F_DST + 1],
                    scalar2=None, op0=ISEQ)
                m2_ps = pm.tile([P, C2], FP, tag="m")
                nc.tensor.matmul(m2_ps[:], lhsT=eat_t[:, c * P : (c + 1) * P],
                                 rhs=w["We2a"][:],
                                 start=True, stop=True)
                t1 = sp.tile([P, C2], FP, tag="t1")
                nc.vector.tensor_tensor(out=t1[:], in0=xl2g[:, c, :],
                                        in1=xr2g[:, c, :], op=ADD)
                nc.vector.tensor_tensor(out=t1[:], in0=t1[:], in1=m2_ps[:],
                                        op=ADD)
                nc.scalar.activation(m2_v[:, c, :], t1[:], PRELU, alpha=alpha02[:])

            nc.vector.tensor_tensor(
                out=m2_v[:], in0=m2_v[:],
                in1=w["att2r"][:].unsqueeze(1).to_broadcast([P, cpb, C2]),
                op=MULT)
            lg2 = bp.tile([P, cpb], FP, tag="lg2")
            nc.vector.tensor_reduce(out=lg2[:], in_=m2_v[:],
                                    axis=mybir.AxisListType.X, op=ADD)
            x9 = bp.tile([P, cpb, C2 + 1], FP, tag="x9")
            nc.scalar.activation(x9[:, :, C2 : C2 + 1],
                                 lg2[:].unsqueeze(2), EXP)
            nc.vector.tensor_tensor(
                out=x9[:, :, 0:C2], in0=xl2g[:],
                in1=x9[:, :, C2 : C2 + 1].to_broadcast([P, cpb, C2]),
                op=MULT)

            numer2_ps = pa.tile([C2, P], FP, tag="numerT")
            denom2_ps = pa.tile([1, P], FP, tag="denom")
            for c in range(cpb):
                nc.tensor.matmul(numer2_ps[:], lhsT=x9[:, c, 0:C2],
                                 rhs=oh_all[:, c, :],
                                 start=(c == 0), stop=False,
                                 skip_group_check=True)
                nc.tensor.matmul(denom2_ps[:], lhsT=x9[:, c, C2 : C2 + 1],
                                 rhs=oh_all[:, c, :],
                                 start=(c == 0), stop=False,
                                 skip_group_check=True)

            # loop chunk
            ef2_ps = pt.tile([P, C2], FP, tag="tp")
            nc.tensor.matmul(ef2_ps[:],
                             lhsT=easum_all[:, b * P : (b + 1) * P],
                             rhs=w["We2a"][:], start=True, stop=True)
            xl2o = sp.tile([P, C2], FP, tag="xl2o")
            nc.sync.dma_start(xl2o[:], xl2loc_d[b * P : (b + 1) * P, :])
            xr2o = sp.tile([P, C2], FP, tag="xr2o")
            nc.sync.dma_start(xr2o[:], xr2loc_d[b * P : (b + 1) * P, :])
            m2l = sp.tile([P, C2], FP, tag="m2l")
            nc.vector.tensor_scalar(out=m2l[:], in0=ef2_ps[:],
                                    scalar1=rdt[:, b : b + 1], scalar2=None,
                                    op0=MULT)
            nc.vector.tensor_tensor(out=m2l[:], in0=m2l[:], in1=xl2o[:], op=ADD)
            nc.vector.tensor_tensor(out=m2l[:], in0=m2l[:], in1=xr2o[:], op=ADD)
            nc.scalar.activation(m2l[:], m2l[:], PRELU, alpha=alpha02[:])
            nc.vector.tensor_tensor(out=m2l[:], in0=m2l[:], in1=w["att2r"][:],
                                    op=MULT)
            x9l = sp.tile([P, C2 + 1], FP, tag="x9l")
            nc.vector.tensor_reduce(out=x9l[:, C2 : C2 + 1], in_=m2l[:],
                                    axis=mybir.AxisListType.X, op=ADD)
            nc.scalar.activation(x9l[:, C2 : C2 + 1], x9l[:, C2 : C2 + 1], EXP)
            nc.vector.tensor_scalar(out=x9l[:, 0:C2], in0=xl2o[:],
                                    scalar1=x9l[:, C2 : C2 + 1], scalar2=None,
                                    op0=MULT)
            nc.tensor.matmul(numer2_ps[:], lhsT=x9l[:, 0:C2], rhs=ident[:],
                             start=False, stop=True, skip_group_check=True)
            nc.tensor.matmul(denom2_ps[:], lhsT=x9l[:, C2 : C2 + 1],
                             rhs=ident[:],
                             start=False, stop=True, skip_group_check=True)

            # finalize
            rc2 = sp.tile([1, P], FP, tag="rc2")
            nc.vector.reciprocal(rc2[:], denom2_ps[:])
            r2f_ps = pt.tile([C2, P], FP, tag="tp")
            nc.tensor.matmul(r2f_ps[:], lhsT=w["ones18"][:], rhs=rc2[:],
                             start=True, stop=True)
            r2f = sp.tile([C2, P], FP, tag="r2f")
            nc.scalar.activation(r2f[:], r2f_ps[:], COPY)
            o2 = sp.tile([C2, P], FP, tag="o2")
            nc.vector.tensor_tensor(out=o2[:], in0=numer2_ps[:],
                                    in1=r2f[:], op=MULT)
            t2m = sp.tile([C2, P], FP, tag="t2m")
            nc.vector.tensor_scalar(out=t2m[:], in0=o2[:],
                                    scalar1=w["bias2e"][:], scalar2=0.0,
                                    op0=ADD, op1=MIN)
            u2 = sp.tile([C2, P], FP, tag="u2")
            nc.scalar.activation(u2[:], t2m[:], EXP)
            r2 = sp.tile([C2, P], FP, tag="r2")
            nc.scalar.activation(r2[:], o2[:], RELU, bias=w["bias2e"][:])
            nc.vector.tensor_tensor(out=o2[:], in0=r2[:], in1=u2[:], op=ADD)
            nc.vector.tensor_scalar(out=o2[:], in0=o2[:], scalar1=-1.0,
                                    scalar2=None, op0=ADD)
            ylin_ps = pt.tile([C2, P], FP, tag="tp")
            nc.tensor.matmul(ylin_ps[:], lhsT=w["Wlin"][:], rhs=o2[:],
                             start=True, stop=True)
            nc.scalar.activation(ylin_all[:, b * P : (b + 1) * P],
                                 ylin_ps[:], COPY)

        # ---------------- output ----------------
        ysig = cp.tile([C2, npc * P], FP)
        nc.scalar.activation(ysig[:], ylin_all[:], SIGM, bias=w["blin"][:])
        for b in range(npc):
            yt_ps = pt.tile([P, C2], FP, tag="tp")
            nc.tensor.transpose(out=yt_ps[:],
                                in_=ysig[:, b * P : (b + 1) * P],
                                identity=ident[0:C2, 0:C2])
            yt = sp.tile([P, C2], FP, tag="yt")
            nc.vector.tensor_copy(yt[:], yt_ps[:])
            nc.sync.dma_start(y_d[b * P : (b + 1) * P, :], yt[:])
    return nc


# --------------------------------------------------------------------------
# runners
# --------------------------------------------------------------------------

def make_in_maps(pp, wp, npc):
    n_pad, nb = pp["n_pad"], pp["nb"]
    xp, rec, rdt = pp["xp"], pp["rec"], pp["rdt"]
    in_maps = []
    for c in range(NCORES):
        m = dict(
            xp=xp,
            xown=xp[c * npc * P : (c + 1) * npc * P],
            rec=np.ascontiguousarray(
                rec[c * npc : (c + 1) * npc].reshape(npc, P, -1)),
            rdt=np.ascontiguousarray(rdt[:, c * npc : (c + 1) * npc]),
            eat=np.ascontiguousarray(pp["eat"][c * npc : (c + 1) * npc]),
        )
        m.update(wp)
        in_maps.append(m)
    return in_maps


def run_graph(inputs, npc, backend="hw", trace=False, debug=False):
    """Full pipeline: prep on host, run on 8 cores, unpermute."""
    x = np.asarray(inputs["x"], np.float32)
    n = x.shape[0]
    pp = prep(x, inputs["edge_index"], inputs["edge_attr"], npc)
    wp = prep_weights(inputs)
    nc = build_nc(npc, pp["cpb"], pp["n_pad"], debug=debug)
    nc.compile()
    in_maps = make_in_maps(pp, wp, npc)
    info = {}
    if backend == "sim":
        from concourse.bass_interp import MultiCoreSim
        sim = MultiCoreSim(nc, num_cores=NCORES,
                           require_finite=False, require_nnan=False)
        for c in range(NCORES):
            core = sim.cores[c]
            for k, v in in_maps[c].items():
                core.tensor(k)[:] = v
        sim.simulate()
        outs = [sim.cores[c].tensor("y") for c in range(NCORES)]
    else:
        from concourse.bass_utils import run_bass_kernel_spmd
        res = run_bass_kernel_spmd(nc, in_maps, list(range(NCORES)),
                                   trace=trace)
        outs = [res.results[c]["y"] for c in range(NCORES)]
        if debug:
            info["xl2loc"] = np.concatenate(
                [res.results[c]["xl2dbg"] for c in range(NCORES)], axis=0)
        info["exec_time_ns"] = res.exec_time_ns
        info["profile_json"] = getattr(res, "profile_json", None)
    yp = np.concatenate(outs, axis=0)         # [n_pad, OUT] permuted order
    y = yp[pp["permpos"][:n]]
    return np.ascontiguousarray(y), info


def kernel(**inputs):
    y, _ = run_graph(inputs, npc=49, backend="hw")
    return y
